# revision 7
# baseline (speedup 1.0000x reference)
"""Trainium2 Bass kernel for nn_EvalMultiModel (gnn_message_passing, 8 cores).

Sharding: derivation-node axis K split 8 ways (512 nodes/level/core), both
ensemble members on every core.  The embedding store holds rows [m0|m1]
(256 bf16 = 512 B) in Shared DRAM; each level's new rows are exchanged with an
8-rank AllGather that writes straight into the store slice for that level.
Parent rows are fetched with indirect DMA (128 rows/call, int32 row indices,
compile-time-specialized on the `parents` input), transposed on the tensor
engine to feature-major, then per rule:
    L1: h^T = W1_r^T x^T   (W1 stationary bf16, fp32 PSUM), fused relu
    L2: streams [W2_r | W2_r w_eval] against relu(h)^T as stationary, so the
        new embedding comes out node-major with the eval logit as a free
        129th column.
The weighted-BCE / posOK / negOK reductions run on device per core; the host
sums the eight partial results.  Host-side python only prepares integer
index/schedule tables and the loss coefficient vectors.
"""
import os
import sys

if "/opt/trn_rl_repo" not in sys.path:
    sys.path.insert(0, "/opt/trn_rl_repo")

import numpy as np

import concourse.bass as bass
import concourse.bacc as bacc
import concourse.mybir as mybir
import concourse.tile as tile
from concourse.bass_utils import run_bass_kernel_spmd
from concourse.masks import make_identity
from bass_rust import add_dep_helper

M, D, R = 2, 128, 8
N0, L, K = 8192, 32, 4096
POS_WEIGHT_EXTRA = 1.0
NC_ = 8
KC = K // NC_              # 512 real nodes per core per level
P = 128
INITC = N0 // NC_          # 1024 init nodes per core
IBLK = INITC // P          # 8
GCALLS = 9                 # indirect gather calls per level (1152 padded rows)
VCOLS = IBLK + L * R       # 264 vals columns per m

f32 = mybir.dt.float32
bf16 = mybir.dt.bfloat16
i32 = mybir.dt.int32
AF = mybir.ActivationFunctionType
ALU = mybir.AluOpType
AX = mybir.AxisListType


def _host_prep(thax_ids, sine_ids, parents, rule_ids, pos_cnt, neg_cnt):
    thax_ids = np.asarray(thax_ids); sine_ids = np.asarray(sine_ids)
    parents = np.asarray(parents); rule_ids = np.asarray(rule_ids)
    pos_cnt = np.asarray(pos_cnt, np.float64); neg_cnt = np.asarray(neg_cnt, np.float64)

    # uniform per-level schedule: u[l, r] slots of rule r on every core
    u = np.zeros((L, R), np.int64)
    gof = np.zeros((L, R + 1), np.int64)
    S = np.zeros(L, np.int64)
    for l in range(L):
        cnt = np.bincount(rule_ids[l], minlength=R)
        u[l] = -(-cnt // NC_)
        assert u[l].max() <= P, f"rule group too large at level {l}"
        gof[l, 1:] = np.cumsum(u[l])
        S[l] = gof[l, R]
        assert 2 * S[l] <= GCALLS * P
    NB = np.zeros(L + 1, np.int64)
    NB[0] = N0
    for l in range(L):
        NB[l + 1] = NB[l] + NC_ * S[l]
    NSTORE = int(NB[L])

    Smax = int(S.max())
    assign = np.full((L, NC_, Smax), -1, np.int64)
    slabpos = np.zeros((L, K), np.int64)
    for l in range(L):
        order = np.argsort(rule_ids[l], kind="stable")
        rs = rule_ids[l][order]
        starts = np.searchsorted(rs, np.arange(R + 1))
        for r in range(R):
            ids = order[starts[r]:starts[r + 1]]
            c = np.arange(len(ids)) % NC_
            q = np.arange(len(ids)) // NC_
            assign[l, c, gof[l, r] + q] = ids
            slabpos[l, ids] = c * S[l] + gof[l, r] + q

    def row_of(nids):
        nids = np.asarray(nids)
        lev = np.clip((nids - N0) // K, 0, L - 1)
        off = (nids - N0) % K
        return np.where(nids < N0, np.maximum(nids, 0),
                        NB[lev] + slabpos[lev, off]).astype(np.int32)

    gidx = np.zeros((NC_, L, P, GCALLS), np.int32)
    for l in range(L):
        Sl = int(S[l])
        for c in range(NC_):
            sl = assign[l, c, :Sl]
            safe = np.maximum(sl, 0)
            p0 = row_of(parents[l, safe, 0])
            p1 = row_of(parents[l, safe, 1])
            p0[sl < 0] = 0
            p1[sl < 0] = 0
            lst = np.zeros(GCALLS * P, np.int32)
            lst[:Sl] = p0
            lst[Sl:2 * Sl] = p1
            gidx[c, l] = lst.reshape(GCALLS, P).T

    tidx = np.zeros((NC_, P, IBLK), np.int32)
    sval = np.zeros((NC_, P, IBLK), np.float32)
    for c in range(NC_):
        mine = np.arange(c * INITC, (c + 1) * INITC)
        tidx[c] = thax_ids[mine].reshape(IBLK, P).T
        sval[c] = sine_ids[mine].astype(np.float32).reshape(IBLK, P).T

    cnt = pos_cnt + neg_cnt
    mask = (cnt > 0).astype(np.float64)
    gold = np.where(cnt > 0, pos_cnt / np.maximum(cnt, 1.0), 0.0)
    tp_, tn_ = pos_cnt.sum(), neg_cnt.sum()
    pw = POS_WEIGHT_EXTRA * tn_ / max(tp_, 1.0) if tp_ > 0 else 1.0
    a = pw * gold * mask * cnt
    ab = a + (1.0 - gold) * mask * cnt
    mpos = mask * pos_cnt
    mneg = mask * neg_cnt

    co = np.zeros((NC_, 4, P, VCOLS), np.float32)
    for c in range(NC_):
        mine = np.arange(c * INITC, (c + 1) * INITC)
        for j, arr in enumerate((ab, a, mpos, mneg)):
            co[c, j, :, :IBLK] = arr[mine].reshape(IBLK, P).T
        for l in range(L):
            sl = assign[l, c, :int(S[l])]
            nid = N0 + l * K + np.maximum(sl, 0)
            for j, arr in enumerate((ab, a, mpos, mneg)):
                v = np.where(sl >= 0, arr[nid], 0.0)
                for r in range(R):
                    s0, s1 = int(gof[l, r]), int(gof[l, r + 1])
                    co[c, j, 0:s1 - s0, IBLK + l * R + r] = v[s0:s1]
    return dict(S=S, gof=gof, NB=NB, NSTORE=NSTORE, gidx=gidx, tidx=tidx,
                sval=sval, co=co, mneg_total=float(mneg.sum()))


def _build(prep, zero_b1, zero_b2, b_eval_vals):
    S, gof, NB, NSTORE = prep["S"], prep["gof"], prep["NB"], prep["NSTORE"]
    nc = bacc.Bacc("TRN2", target_bir_lowering=False, debug=False, num_devices=NC_)

    def dt_in(n, s, d=f32):
        return nc.dram_tensor(n, s, d, kind="ExternalInput").ap()

    gidx = dt_in("gidx", [L, P, GCALLS], i32)
    tidx = dt_in("tidx", [P, IBLK], i32)
    sval = dt_in("sval", [P, IBLK])
    co = dt_in("co", [4, P, VCOLS])
    thax0 = dt_in("thax0", [1000, D])
    thax1 = dt_in("thax1", [1000, D])
    W1 = dt_in("W1", [M, R, 2 * D, D])
    W2 = dt_in("W2", [M, R, D, D])
    b1 = dt_in("b1", [M, R, D])
    b2 = dt_in("b2", [M, R, D])
    sine_w = dt_in("sine_w", [M, D])
    sine_b = dt_in("sine_b", [M, D])
    w_eval = dt_in("w_eval", [M, D])
    out = nc.dram_tensor("out", [8], f32, kind="ExternalOutput").ap()
    store = nc.dram_tensor("store", [NSTORE, 2 * D], bf16, addr_space="Shared").ap()

    with tile.TileContext(nc) as tc:
        with (
            tc.tile_pool(name="persist", bufs=1) as persist,
            tc.tile_pool(name="initp", bufs=1) as initp,
            tc.tile_pool(name="work", bufs=2) as pool,
            tc.tile_pool(name="gt", bufs=3) as gpool,
            tc.tile_pool(name="tp", bufs=2, space="PSUM") as tp,
            tc.tile_pool(name="hp", bufs=2, space="PSUM") as hp,
            tc.tile_pool(name="l2p", bufs=2, space="PSUM") as l2p,
            tc.tile_pool(name="dram", bufs=2, space="DRAM") as dpool,
        ):
            # ---------------- one-time prep ----------------
            ident = persist.tile([P, P], bf16)
            make_identity(nc, ident[:])
            onesc = persist.tile([1, P], f32)
            nc.vector.memset(onesc[:], 1.0)
            ones_col = persist.tile([P, 1], f32)
            nc.vector.memset(ones_col[:], 1.0)

            idxt = persist.tile([P, L, GCALLS], i32)
            nc.sync.dma_start(out=idxt[:], in_=gidx.rearrange("l p g -> p l g"))
            tidxt = persist.tile([P, IBLK], i32)
            nc.sync.dma_start(out=tidxt[:], in_=tidx[:])
            cot = persist.tile([P, 4, VCOLS], f32)
            nc.sync.dma_start(out=cot[:], in_=co.rearrange("j p v -> p j v"))

            w1f = initp.tile([P, M * R * 2, D], f32)
            nc.sync.dma_start(out=w1f[:], in_=W1.rearrange("m r (t p) e -> p (m r t) e", p=P))
            w1sb = persist.tile([P, M * R * 2, D], bf16)
            nc.vector.tensor_copy(out=w1sb[:], in_=w1f[:])

            w2f = initp.tile([P, M * R, D], f32)
            nc.sync.dma_start(out=w2f[:], in_=W2.rearrange("m r e f -> e (m r) f"))

            b1t = persist.tile([P, M * R], f32)
            nc.sync.dma_start(out=b1t[:], in_=b1.rearrange("m r d -> d (m r)"))

            brows = initp.tile([1, 6, D], f32)
            nc.sync.dma_start(out=brows[:, 0:2, :], in_=sine_w.rearrange("m d -> m d")[None])
            nc.sync.dma_start(out=brows[:, 2:4, :], in_=sine_b[None])
            nc.sync.dma_start(out=brows[:, 4:6, :], in_=w_eval[None])
            bc = persist.tile([P, 6, D], f32)  # w_m0 w_m1 b_m0 b_m1 ev_m0 ev_m1
            for j in range(6):
                bps = tp.tile([P, 512], f32, tag="tps")
                nc.tensor.matmul(out=bps[:, 0:P], lhsT=onesc[:], rhs=brows[:, j, :],
                                 start=True, stop=True)
                nc.vector.tensor_copy(out=bc[:, j, :], in_=bps[:, 0:P])

            # b2 broadcast rows (only if nonzero): [P, (m r), D]
            if not zero_b2:
                b2r = initp.tile([1, M * R, D], f32)
                nc.sync.dma_start(out=b2r[:], in_=b2.rearrange("m r d -> (m r) d")[None])
                b2bc = persist.tile([P, M * R, D], f32)
                for j in range(M * R):
                    bps = tp.tile([P, 512], f32, tag="tps")
                    nc.tensor.matmul(out=bps[:, 0:P], lhsT=onesc[:], rhs=b2r[:, j, :],
                                     start=True, stop=True)
                    nc.vector.tensor_copy(out=b2bc[:, j, :], in_=bps[:, 0:P])

            # B' = [W2_mr (+b2 fold NO) | W2_mr @ w_eval_m] bf16
            bp = persist.tile([P, M * R, D + 1], bf16)
            for m in range(M):
                for r in range(R):
                    i = m * R + r
                    t = initp.tile([P, D], f32, tag="w2e")
                    nc.vector.tensor_tensor(out=t[:], in0=w2f[:, i, :], in1=bc[:, 4 + m, :], op=ALU.mult)
                    t2 = initp.tile([P, 1], f32, tag="w2e2")
                    nc.vector.reduce_sum(out=t2[:], in_=t[:], axis=AX.X)
                    nc.scalar.copy(out=bp[:, i, 0:D], in_=w2f[:, i, :])
                    nc.vector.tensor_copy(out=bp[:, i, D:D + 1], in_=t2[:])

            # ---------------- init nodes ----------------
            vals0 = persist.tile([P, VCOLS], f32)
            vals1 = persist.tile([P, VCOLS], f32)
            valsm = [vals0, vals1]
            svt = persist.tile([P, IBLK], f32)
            nc.sync.dma_start(out=svt[:], in_=sval[:])
            islab = initp.tile([P, IBLK, 2 * D], bf16)
            for m in range(M):
                e = initp.tile([P, IBLK, D], f32, tag=f"emb{m}")
                th = (thax0, thax1)[m]
                for g in range(IBLK):
                    nc.gpsimd.indirect_dma_start(
                        out=e[:, g, :], out_offset=None, in_=th[:],
                        in_offset=bass.IndirectOffsetOnAxis(ap=tidxt[:, g:g + 1], axis=0))
                for g in range(IBLK):
                    t = initp.tile([P, D], f32, tag="sig")
                    nc.vector.tensor_tensor(out=t[:], in0=svt[:, g:g + 1].to_broadcast([P, D]),
                                            in1=bc[:, m, :], op=ALU.mult)
                    nc.vector.tensor_tensor(out=t[:], in0=t[:], in1=bc[:, 2 + m, :], op=ALU.add)
                    nc.scalar.activation(out=t[:], in_=t[:], func=AF.Sigmoid)
                    nc.vector.tensor_tensor(out=e[:, g, :], in0=e[:, g, :], in1=t[:], op=ALU.mult)
                    t2 = initp.tile([P, D], f32, tag="sig2")
                    nc.vector.tensor_tensor(out=t2[:], in0=e[:, g, :], in1=bc[:, 4 + m, :], op=ALU.mult)
                    nc.vector.reduce_sum(out=valsm[m][:, g:g + 1], in_=t2[:], axis=AX.X)
                nc.scalar.copy(out=islab[:, :, m * D:(m + 1) * D], in_=e[:])
            ibounce = dpool.tile([INITC, 2 * D], bf16, tag="ibounce")
            nc.sync.dma_start(out=ibounce[:].rearrange("(a p) r -> p a r", p=P), in_=islab[:])
            prev_ag = nc.gpsimd.collective_compute(
                "AllGather", ALU.bypass, replica_groups=[list(range(NC_))],
                ins=[ibounce[:]], outs=[store[0:N0]])
            tc.dep_state.clear_tensor_accesses(store.tensor.name)

            # ---------------- levels ----------------
            for l in range(L):
                Sl = int(S[l])
                gt = gpool.tile([P, GCALLS, 2 * D], bf16, tag="gt")
                for g in range(GCALLS):
                    gi = nc.gpsimd.indirect_dma_start(
                        out=gt[:, g, :], out_offset=None, in_=store[:],
                        in_offset=bass.IndirectOffsetOnAxis(ap=idxt[:, l, g:g + 1], axis=0))
                    add_dep_helper(gi.ins, prev_ag.ins, reason=f"gather l{l}")

                xt = [pool.tile([P, GCALLS * P], bf16, tag=f"xt{m}", name=f"xt{m}_{l}") for m in range(M)]
                for m in range(M):
                    for blk in range(3):  # 4 + 4 + 1 slots
                        n_s = 4 if blk < 2 else 1
                        ps = tp.tile([P, 512], bf16, tag="tpsb")
                        for k in range(n_s):
                            s = blk * 4 + k
                            nc.tensor.transpose(out=ps[:, k * P:(k + 1) * P],
                                                in_=gt[:, s, m * D:(m + 1) * D],
                                                identity=ident[:])
                        if (m + blk) % 2:
                            nc.scalar.copy(out=xt[m][:, blk * 512: blk * 512 + n_s * P],
                                           in_=ps[:, 0:n_s * P])
                        else:
                            nc.vector.tensor_copy(out=xt[m][:, blk * 512: blk * 512 + n_s * P],
                                                  in_=ps[:, 0:n_s * P])

                hrelu = [pool.tile([P, 640], bf16, tag=f"hr{m}", name=f"hr{m}_{l}") for m in range(M)]
                for m in range(M):
                    nc.vector.memset(hrelu[m][:], 0.0)
                    for half in range(2):
                        r0, r1 = (0, 4) if half == 0 else (4, 8)
                        c0 = int(gof[l, r0])
                        width = int(gof[l, r1]) - c0
                        if width == 0:
                            continue
                        hps = hp.tile([P, 512], f32, tag="hps")
                        for r in range(r0, r1):
                            a_r, b_r = int(gof[l, r]), int(gof[l, r + 1])
                            if b_r == a_r:
                                continue
                            w1i = (m * R + r) * 2
                            nc.tensor.matmul(out=hps[:, a_r - c0:b_r - c0],
                                             lhsT=w1sb[:, w1i, :],
                                             rhs=xt[m][:, a_r:b_r], start=True, stop=False)
                            nc.tensor.matmul(out=hps[:, a_r - c0:b_r - c0],
                                             lhsT=w1sb[:, w1i + 1, :],
                                             rhs=xt[m][:, Sl + a_r:Sl + b_r],
                                             start=False, stop=True)
                        if zero_b1:
                            nc.scalar.activation(out=hrelu[m][:, c0:c0 + width],
                                                 in_=hps[:, 0:width], func=AF.Relu)
                        else:
                            for r in range(r0, r1):
                                a_r, b_r = int(gof[l, r]), int(gof[l, r + 1])
                                if b_r == a_r:
                                    continue
                                nc.scalar.activation(out=hrelu[m][:, a_r:b_r],
                                                     in_=hps[:, a_r - c0:b_r - c0],
                                                     func=AF.Relu,
                                                     bias=b1t[:, m * R + r: m * R + r + 1])

                slab = pool.tile([P, 5, 2 * D], bf16, tag="slab")
                for m in range(M):
                    for bank in range(3):
                        rr = list(range(bank * 3, min(R, bank * 3 + 3)))
                        l2 = l2p.tile([P, 512], f32, tag="l2ps")
                        for j, r in enumerate(rr):
                            a_r = int(gof[l, r])
                            nc.tensor.matmul(out=l2[:, j * 129:(j + 1) * 129],
                                             lhsT=hrelu[m][:, a_r:a_r + P],
                                             rhs=bp[:, m * R + r, :], start=True, stop=True)
                        if not zero_b2:
                            for j, r in enumerate(rr):
                                nc.vector.tensor_tensor(
                                    out=l2[:, j * 129:j * 129 + D],
                                    in0=l2[:, j * 129:j * 129 + D],
                                    in1=b2bc[:, m * R + r, :], op=ALU.add)
                        nsb = pool.tile([P, 512], bf16, tag="nsb")
                        if (m + bank) % 2:
                            nc.scalar.copy(out=nsb[:, 0:len(rr) * 129], in_=l2[:, 0:len(rr) * 129])
                        else:
                            nc.vector.tensor_copy(out=nsb[:, 0:len(rr) * 129], in_=l2[:, 0:len(rr) * 129])
                        nc.vector.tensor_copy(
                            out=valsm[m][:, IBLK + l * R + bank * 3:
                                         IBLK + l * R + bank * 3 + len(rr)],
                            in_=l2[:, D:D + 1 + (len(rr) - 1) * 129:129])
                        for j, r in enumerate(rr):
                            u_r = int(gof[l, r + 1]) - int(gof[l, r])
                            if u_r == 0:
                                continue
                            dst0 = int(gof[l, r])
                            done = 0
                            while done < u_r:
                                dp = (dst0 + done) % P
                                slot = (dst0 + done) // P
                                n = min(u_r - done, P - dp)
                                nc.sync.dma_start(
                                    out=slab[dp:dp + n, slot, m * D:(m + 1) * D],
                                    in_=nsb[done:done + n, j * 129:j * 129 + D])
                                done += n

                sbounce = dpool.tile([5 * P, 2 * D], bf16, tag="sbounce")
                nc.sync.dma_start(out=sbounce[:].rearrange("(a p) r -> p a r", p=P),
                                  in_=slab[:])
                base = int(NB[l])
                prev_ag = nc.gpsimd.collective_compute(
                    "AllGather", ALU.bypass, replica_groups=[list(range(NC_))],
                    ins=[sbounce[0:Sl]], outs=[store[base: base + NC_ * Sl]])
                tc.dep_state.clear_tensor_accesses(store.tensor.name)

            # ---------------- BCE tail ----------------
            acc = persist.tile([P, 6], f32)
            for m in range(M):
                v = valsm[m]
                if b_eval_vals[m] != 0.0:
                    nc.vector.tensor_scalar(out=v[:], in0=v[:], scalar1=float(b_eval_vals[m]),
                                            scalar2=None, op0=ALU.add)
                sp = pool.tile([P, VCOLS], f32, tag="sp")
                nc.scalar.activation(out=sp[:], in_=v[:], func=AF.Abs)
                nc.scalar.activation(out=sp[:], in_=sp[:], func=AF.Exp, scale=-1.0)
                nc.scalar.activation(out=sp[:], in_=sp[:], func=AF.Ln, bias=1.0)
                rl = pool.tile([P, VCOLS], f32, tag="rl")
                nc.scalar.activation(out=rl[:], in_=v[:], func=AF.Relu)
                nc.vector.tensor_tensor(out=sp[:], in0=sp[:], in1=rl[:], op=ALU.add)
                t = pool.tile([P, VCOLS], f32, tag="bce")
                nc.vector.tensor_tensor(out=t[:], in0=sp[:], in1=cot[:, 0, :], op=ALU.mult)
                t2 = pool.tile([P, VCOLS], f32, tag="bce2")
                nc.vector.tensor_tensor(out=t2[:], in0=v[:], in1=cot[:, 1, :], op=ALU.mult)
                nc.vector.tensor_tensor(out=t[:], in0=t[:], in1=t2[:], op=ALU.subtract)
                nc.vector.reduce_sum(out=acc[:, m:m + 1], in_=t[:], axis=AX.X)
                ge = pool.tile([P, VCOLS], f32, tag="ge")
                nc.vector.tensor_scalar(out=ge[:], in0=v[:], scalar1=0.0, scalar2=None,
                                        op0=ALU.is_ge)
                nc.vector.tensor_tensor(out=t[:], in0=ge[:], in1=cot[:, 2, :], op=ALU.mult)
                nc.vector.reduce_sum(out=acc[:, 2 + m:3 + m], in_=t[:], axis=AX.X)
                nc.vector.tensor_tensor(out=t[:], in0=ge[:], in1=cot[:, 3, :], op=ALU.mult)
                nc.vector.reduce_sum(out=acc[:, 4 + m:5 + m], in_=t[:], axis=AX.X)
            rps = tp.tile([P, 512], f32, tag="tps")
            nc.tensor.matmul(out=rps[0:6, 0:1], lhsT=acc[:], rhs=ones_col[:],
                             start=True, stop=True)
            outt = pool.tile([6, 1], f32, tag="outt")
            nc.vector.tensor_copy(out=outt[:], in_=rps[0:6, 0:1])
            nc.sync.dma_start(out=out[0:6, None], in_=outt[:])

    nc.compile()
    return nc


def kernel(thax_ids, sine_ids, parents, rule_ids, pos_cnt, neg_cnt,
           thax_table, sine_w, sine_b, W1, b1, W2, b2, w_eval, b_eval):
    prep = _host_prep(thax_ids, sine_ids, parents, rule_ids, pos_cnt, neg_cnt)
    zero_b1 = not np.any(np.asarray(b1))
    zero_b2 = not np.any(np.asarray(b2))
    b_eval_vals = [float(x) for x in np.asarray(b_eval)]
    nc = _build(prep, zero_b1, zero_b2, b_eval_vals)

    common = dict(
        thax0=np.ascontiguousarray(np.asarray(thax_table, np.float32)[0]),
        thax1=np.ascontiguousarray(np.asarray(thax_table, np.float32)[1]),
        W1=np.ascontiguousarray(np.asarray(W1, np.float32)),
        W2=np.ascontiguousarray(np.asarray(W2, np.float32)),
        b1=np.ascontiguousarray(np.asarray(b1, np.float32)),
        b2=np.ascontiguousarray(np.asarray(b2, np.float32)),
        sine_w=np.ascontiguousarray(np.asarray(sine_w, np.float32)),
        sine_b=np.ascontiguousarray(np.asarray(sine_b, np.float32)),
        w_eval=np.ascontiguousarray(np.asarray(w_eval, np.float32)),
    )
    in_maps = []
    for c in range(NC_):
        in_maps.append(dict(common,
                            gidx=prep["gidx"][c], tidx=prep["tidx"][c],
                            sval=prep["sval"][c], co=prep["co"][c]))
    trace = os.environ.get("KTRACE", "0") == "1"
    res = run_bass_kernel_spmd(nc, in_maps, core_ids=list(range(NC_)), trace=trace)
    if trace and res.exec_time_ns is not None:
        print(f"HW exec time: {res.exec_time_ns} ns")

    loss = np.zeros(M, np.float64)
    posOK = np.zeros(M, np.float64)
    negge = np.zeros(M, np.float64)
    for c in range(NC_):
        o = np.asarray(res.results[c]["out"], np.float64)
        loss += o[0:2]
        posOK += o[2:4]
        negge += o[4:6]
    negOK = prep["mneg_total"] - negge
    return np.stack([loss, posOK, negOK]).astype(np.float32)


# revision 14
# speedup vs baseline: 1.0700x; 1.0700x over previous
"""Trainium2 Bass kernel for nn_EvalMultiModel (gnn_message_passing, 8 cores).

Sharding: derivation-node axis K split 8 ways (512 nodes/level/core), both
ensemble members on every core.  The embedding store holds rows [m0|m1]
(256 bf16 = 512 B) in Shared DRAM; each level's new rows are exchanged with an
8-rank AllGather that writes straight into the store slice for that level.
Parent rows are fetched with indirect DMA (128 rows/call, int32 row indices,
compile-time-specialized on the `parents` input), transposed on the tensor
engine to feature-major, then per rule:
    L1: h^T = W1_r^T x^T   (W1 stationary bf16, fp32 PSUM), fused relu
    L2: streams [W2_r | W2_r w_eval] against relu(h)^T as stationary, so the
        new embedding comes out node-major with the eval logit as a free
        129th column.
The weighted-BCE / posOK / negOK reductions run on device per core; the host
sums the eight partial results.  Host-side python only prepares integer
index/schedule tables and the loss coefficient vectors.
"""
import os
import sys

if "/opt/trn_rl_repo" not in sys.path:
    sys.path.insert(0, "/opt/trn_rl_repo")

import numpy as np

import concourse.bass as bass
import concourse.bacc as bacc
import concourse.mybir as mybir
import concourse.tile as tile
from concourse.bass_utils import run_bass_kernel_spmd
from concourse.masks import make_identity
from bass_rust import add_dep_helper

M, D, R = 2, 128, 8
N0, L, K = 8192, 32, 4096
POS_WEIGHT_EXTRA = 1.0
NC_ = 8
KC = K // NC_              # 512 real nodes per core per level
P = 128
INITC = N0 // NC_          # 1024 init nodes per core
IBLK = INITC // P          # 8
GCALLS = 9                 # indirect gather calls per level (1152 padded rows)
VCOLS = IBLK + L * R       # 264 vals columns per m

f32 = mybir.dt.float32
bf16 = mybir.dt.bfloat16
i32 = mybir.dt.int32
AF = mybir.ActivationFunctionType
ALU = mybir.AluOpType
AX = mybir.AxisListType


def _host_prep(thax_ids, sine_ids, parents, rule_ids, pos_cnt, neg_cnt):
    thax_ids = np.asarray(thax_ids); sine_ids = np.asarray(sine_ids)
    parents = np.asarray(parents); rule_ids = np.asarray(rule_ids)
    pos_cnt = np.asarray(pos_cnt, np.float64); neg_cnt = np.asarray(neg_cnt, np.float64)

    u = np.zeros((L, R), np.int64)
    for l in range(L):
        cnt = np.bincount(rule_ids[l], minlength=R)
        u[l] = -(-cnt // NC_)

    # real assignment: per (level, rule) deal round-robin
    corec = np.zeros((L, K), np.int64)
    coreq = np.zeros((L, K), np.int64)
    reals = [[None] * NC_ for _ in range(L)]   # per (l, c): [R][...] node offsets
    for l in range(L):
        order = np.argsort(rule_ids[l], kind="stable")
        rs = rule_ids[l][order]
        starts = np.searchsorted(rs, np.arange(R + 1))
        for c in range(NC_):
            reals[l][c] = [order[starts[r]:starts[r + 1]][c::NC_] for r in range(R)]
        for r in range(R):
            ids = order[starts[r]:starts[r + 1]]
            corec[l, ids] = np.arange(len(ids)) % NC_
            coreq[l, ids] = np.arange(len(ids)) // NC_

    LEV = NC_ * P * R          # padded real rows per level
    DUPR = P * R               # dup-region rows per level
    NB = np.zeros(L + 1, np.int64)
    NB[0] = N0
    for l in range(L):
        NB[l + 1] = NB[l] + LEV + DUPR
    NSTORE = int(NB[L])

    def real_row(nids):
        nids = np.asarray(nids, np.int64)
        lev = np.clip((nids - N0) // K, 0, L - 1)
        off = (nids - N0) % K
        rr = rule_ids[lev, off]
        pos = NB[lev] + corec[lev, off] * (P * R) + rr * P + coreq[lev, off]
        return np.where(nids < N0, np.maximum(nids, 0), pos)

    # backward closure of dup sets: dup_nodes[lc][c] = level-lc nodes, not owned
    # by c, referenced by core c's batch at level lc+1 (reals + its dups).
    USE_DUPS = False
    dup_nodes = [[np.zeros(0, np.int64) for _ in range(NC_)] for _ in range(L)]
    if USE_DUPS:
        for l in range(L - 1, 0, -1):
            for c in range(NC_):
                roff = np.concatenate(reals[l][c]).astype(np.int64)
                doff = ((dup_nodes[l][c] - N0) % K).astype(np.int64)
                batch = np.concatenate([roff, doff])
                par = parents[l, batch].reshape(-1).astype(np.int64)
                lev = (par - N0) // K
                sel = (par >= N0) & (lev == l - 1)
                tgt = np.unique(par[sel])
                if len(tgt):
                    tgt = tgt[corec[l - 1, (tgt - N0) % K] != c]
                dup_nodes[l - 1][c] = tgt

    d = np.zeros((L, R), np.int64)
    dq = [[None] * NC_ for _ in range(L)]
    for lc in range(L):
        dcnt = np.zeros((NC_, R), np.int64)
        for c in range(NC_):
            tg = dup_nodes[lc][c]
            if len(tg):
                dcnt[c] = np.bincount(rule_ids[lc, (tg - N0) % K], minlength=R)
        d[lc] = dcnt.max(0)
        assert (u[lc] + d[lc]).max() <= P, f"rule block overflow lvl {lc}"
        for c in range(NC_):
            tg = dup_nodes[lc][c]
            mp = {}
            if len(tg):
                rr = rule_ids[lc, (tg - N0) % K]
                o = np.argsort(rr, kind="stable")
                pos = np.zeros(R, np.int64)
                for nid, r in zip(tg[o], rr[o]):
                    mp[int(nid)] = (int(r), int(pos[r]))
                    pos[r] += 1
            dq[lc][c] = mp

    gof = np.zeros((L, R + 1), np.int64)
    S = np.zeros(L, np.int64)
    gcalls = np.zeros(L, np.int64)
    for l in range(L):
        gof[l, 1:] = np.cumsum(u[l] + d[l])
        S[l] = gof[l, R]
        gcalls[l] = -(-2 * S[l] // P)
    goff = np.zeros(L + 1, np.int64)
    for l in range(L):
        goff[l + 1] = goff[l] + gcalls[l]
    TOTG = int(goff[L])

    def ref_row(par, l, c):
        par = np.asarray(par, np.int64)
        out = real_row(par)
        if not USE_DUPS:
            return out
        lev = (par - N0) // K
        sel = (par >= N0) & (lev == l - 1)
        for i in np.nonzero(sel)[0]:
            nid = int(par[i])
            if corec[l - 1, (nid - N0) % K] != c:
                r, j = dq[l - 1][c][nid]
                out[i] = NB[l - 1] + LEV + r * P + int(u[l - 1, r]) + j
        return out

    # slot lists, gather indices
    gidx = np.zeros((NC_, P, TOTG), np.int32)
    Smax = int(S.max())
    slotnode = np.full((L, NC_, Smax), -1, np.int64)
    for l in range(L):
        Sl = int(S[l])
        for c in range(NC_):
            for r in range(R):
                base = int(gof[l, r])
                ids = reals[l][c][r]
                for q, nid in enumerate(ids):
                    slotnode[l, c, base + q] = N0 + l * K + nid
            for nid, (r, j) in dq[l][c].items():
                slotnode[l, c, int(gof[l, r]) + int(u[l, r]) + j] = nid
            sn = slotnode[l, c, :Sl]
            ok = sn >= 0
            off = np.where(ok, (sn - N0) % K, 0)
            pn = parents[l, off].astype(np.int64)
            pn[~ok] = 0
            lst = np.zeros(int(gcalls[l]) * P, np.int64)
            lst[:Sl] = np.where(ok, ref_row(pn[:, 0], l, c), 0)
            lst[Sl:2 * Sl] = np.where(ok, ref_row(pn[:, 1], l, c), 0)
            gidx[c, :, int(goff[l]):int(goff[l + 1])] = \
                lst.reshape(int(gcalls[l]), P).T.astype(np.int32)

    tidx = np.zeros((NC_, P, IBLK), np.int32)
    sval = np.zeros((NC_, P, IBLK), np.float32)
    for c in range(NC_):
        mine = np.arange(c * INITC, (c + 1) * INITC)
        tidx[c] = thax_ids[mine].reshape(IBLK, P).T
        sval[c] = sine_ids[mine].astype(np.float32).reshape(IBLK, P).T

    cnt = pos_cnt + neg_cnt
    mask = (cnt > 0).astype(np.float64)
    gold = np.where(cnt > 0, pos_cnt / np.maximum(cnt, 1.0), 0.0)
    tp_, tn_ = pos_cnt.sum(), neg_cnt.sum()
    pw = POS_WEIGHT_EXTRA * tn_ / max(tp_, 1.0) if tp_ > 0 else 1.0
    a = pw * gold * mask * cnt
    ab = a + (1.0 - gold) * mask * cnt
    mpos = mask * pos_cnt
    mneg = mask * neg_cnt

    co = np.zeros((NC_, 4, P, VCOLS), np.float32)
    for c in range(NC_):
        mine = np.arange(c * INITC, (c + 1) * INITC)
        for j, arr in enumerate((ab, a, mpos, mneg)):
            co[c, j, :, :IBLK] = arr[mine].reshape(IBLK, P).T
        for l in range(L):
            for r in range(R):
                ids = reals[l][c][r]
                nid = N0 + l * K + ids
                for j, arr in enumerate((ab, a, mpos, mneg)):
                    co[c, j, 0:len(ids), IBLK + l * R + r] = arr[nid]
    return dict(S=S, gof=gof, u=u, NB=NB, NSTORE=NSTORE, gidx=gidx, tidx=tidx,
                sval=sval, co=co, mneg_total=float(mneg.sum()),
                gcalls=gcalls, goff=goff, TOTG=TOTG, LEV=LEV)


def _build(prep, zero_b1, zero_b2, b_eval_vals):
    S, gof, NB, NSTORE = prep["S"], prep["gof"], prep["NB"], prep["NSTORE"]
    nc = bacc.Bacc("TRN2", target_bir_lowering=False, debug=False, num_devices=NC_)

    def dt_in(n, s, d=f32):
        return nc.dram_tensor(n, s, d, kind="ExternalInput").ap()

    TOTG = prep["TOTG"]
    gcalls, goff = prep["gcalls"], prep["goff"]
    gidx = dt_in("gidx", [P, TOTG], i32)
    tidx = dt_in("tidx", [P, IBLK], i32)
    sval = dt_in("sval", [P, IBLK])
    co = dt_in("co", [4, P, VCOLS])
    thax0 = dt_in("thax0", [1000, D])
    thax1 = dt_in("thax1", [1000, D])
    W1 = dt_in("W1", [M, R, 2 * D, D])
    W2 = dt_in("W2", [M, R, D, D])
    b1 = dt_in("b1", [M, R, D])
    b2 = dt_in("b2", [M, R, D])
    sine_w = dt_in("sine_w", [M, D])
    sine_b = dt_in("sine_b", [M, D])
    w_eval = dt_in("w_eval", [M, D])
    out = nc.dram_tensor("out", [8], f32, kind="ExternalOutput").ap()
    store = nc.dram_tensor("store", [NSTORE, 2 * D], bf16, addr_space="Shared").ap()

    with tile.TileContext(nc) as tc:
        with (
            tc.tile_pool(name="persist", bufs=1) as persist,
            tc.tile_pool(name="initp", bufs=1) as initp,
            tc.tile_pool(name="work", bufs=2) as pool,
            tc.tile_pool(name="gt", bufs=3) as gpool,
            tc.tile_pool(name="tp", bufs=2, space="PSUM") as tp,
            tc.tile_pool(name="hp", bufs=2, space="PSUM") as hp,
            tc.tile_pool(name="l2p", bufs=2, space="PSUM") as l2p,
            tc.tile_pool(name="dram", bufs=2, space="DRAM") as dpool,
        ):
            # ---------------- one-time prep ----------------
            ident = persist.tile([P, P], bf16)
            make_identity(nc, ident[:])
            onesc = persist.tile([1, P], f32)
            nc.vector.memset(onesc[:], 1.0)
            ones_col = persist.tile([P, 1], f32)
            nc.vector.memset(ones_col[:], 1.0)

            idxt = persist.tile([P, TOTG], i32)
            nc.sync.dma_start(out=idxt[:], in_=gidx[:])
            tidxt = persist.tile([P, IBLK], i32)
            nc.sync.dma_start(out=tidxt[:], in_=tidx[:])
            cot = persist.tile([P, 4, VCOLS], f32)
            nc.sync.dma_start(out=cot[:], in_=co.rearrange("j p v -> p j v"))

            w1f = initp.tile([P, M * R * 2, D], f32)
            nc.sync.dma_start(out=w1f[:], in_=W1.rearrange("m r (t p) e -> p (m r t) e", p=P))
            w1sb = persist.tile([P, M * R * 2, D], bf16)
            nc.vector.tensor_copy(out=w1sb[:], in_=w1f[:])

            w2f = initp.tile([P, M * R, D], f32)
            nc.sync.dma_start(out=w2f[:], in_=W2.rearrange("m r e f -> e (m r) f"))

            b1t = persist.tile([P, M * R], f32)
            nc.sync.dma_start(out=b1t[:], in_=b1.rearrange("m r d -> d (m r)"))

            brows = initp.tile([1, 6, D], f32)
            nc.sync.dma_start(out=brows[:, 0:2, :], in_=sine_w.rearrange("m d -> m d")[None])
            nc.sync.dma_start(out=brows[:, 2:4, :], in_=sine_b[None])
            nc.sync.dma_start(out=brows[:, 4:6, :], in_=w_eval[None])
            bc = persist.tile([P, 6, D], f32)  # w_m0 w_m1 b_m0 b_m1 ev_m0 ev_m1
            for j in range(6):
                bps = tp.tile([P, 512], f32, tag="tps")
                nc.tensor.matmul(out=bps[:, 0:P], lhsT=onesc[:], rhs=brows[:, j, :],
                                 start=True, stop=True)
                nc.vector.tensor_copy(out=bc[:, j, :], in_=bps[:, 0:P])

            # b2 broadcast rows (only if nonzero): [P, (m r), D]
            if not zero_b2:
                b2r = initp.tile([1, M * R, D], f32)
                nc.sync.dma_start(out=b2r[:], in_=b2.rearrange("m r d -> (m r) d")[None])
                b2bc = persist.tile([P, M * R, D], f32)
                for j in range(M * R):
                    bps = tp.tile([P, 512], f32, tag="tps")
                    nc.tensor.matmul(out=bps[:, 0:P], lhsT=onesc[:], rhs=b2r[:, j, :],
                                     start=True, stop=True)
                    nc.vector.tensor_copy(out=b2bc[:, j, :], in_=bps[:, 0:P])

            # B' = [W2_mr (+b2 fold NO) | W2_mr @ w_eval_m] bf16
            bp = persist.tile([P, M * R, D + 1], bf16)
            for m in range(M):
                for r in range(R):
                    i = m * R + r
                    t = initp.tile([P, D], f32, tag="w2e")
                    nc.vector.tensor_tensor(out=t[:], in0=w2f[:, i, :], in1=bc[:, 4 + m, :], op=ALU.mult)
                    t2 = initp.tile([P, 1], f32, tag="w2e2")
                    nc.vector.reduce_sum(out=t2[:], in_=t[:], axis=AX.X)
                    nc.scalar.copy(out=bp[:, i, 0:D], in_=w2f[:, i, :])
                    nc.vector.tensor_copy(out=bp[:, i, D:D + 1], in_=t2[:])

            # ---------------- init nodes ----------------
            vals0 = persist.tile([P, VCOLS], f32)
            vals1 = persist.tile([P, VCOLS], f32)
            valsm = [vals0, vals1]
            svt = persist.tile([P, IBLK], f32)
            nc.sync.dma_start(out=svt[:], in_=sval[:])
            islab = initp.tile([P, IBLK, 2 * D], bf16)
            for m in range(M):
                e = initp.tile([P, IBLK, D], f32, tag=f"emb{m}")
                th = (thax0, thax1)[m]
                for g in range(IBLK):
                    nc.gpsimd.indirect_dma_start(
                        out=e[:, g, :], out_offset=None, in_=th[:],
                        in_offset=bass.IndirectOffsetOnAxis(ap=tidxt[:, g:g + 1], axis=0))
                for g in range(IBLK):
                    t = initp.tile([P, D], f32, tag="sig")
                    nc.vector.tensor_tensor(out=t[:], in0=svt[:, g:g + 1].to_broadcast([P, D]),
                                            in1=bc[:, m, :], op=ALU.mult)
                    nc.vector.tensor_tensor(out=t[:], in0=t[:], in1=bc[:, 2 + m, :], op=ALU.add)
                    nc.scalar.activation(out=t[:], in_=t[:], func=AF.Sigmoid)
                    nc.vector.tensor_tensor(out=e[:, g, :], in0=e[:, g, :], in1=t[:], op=ALU.mult)
                    t2 = initp.tile([P, D], f32, tag="sig2")
                    nc.vector.tensor_tensor(out=t2[:], in0=e[:, g, :], in1=bc[:, 4 + m, :], op=ALU.mult)
                    nc.vector.reduce_sum(out=valsm[m][:, g:g + 1], in_=t2[:], axis=AX.X)
                nc.scalar.copy(out=islab[:, :, m * D:(m + 1) * D], in_=e[:])
            ibounce = dpool.tile([INITC, 2 * D], bf16, tag="ibounce")
            nc.sync.dma_start(out=ibounce[:].rearrange("(a p) r -> p a r", p=P), in_=islab[:])
            prev_ag = nc.gpsimd.collective_compute(
                "AllGather", ALU.bypass, replica_groups=[list(range(NC_))],
                ins=[ibounce[:]], outs=[store[0:N0]])
            tc.dep_state.clear_tensor_accesses(store.tensor.name)

            # ---------------- levels ----------------
            XTW = int(max(gcalls)) * P
            for l in range(L):
                Sl = int(S[l])
                ncall = int(gcalls[l])
                g0 = int(goff[l])
                gt = gpool.tile([P, ncall, 2 * D], bf16, tag="gt", name=f"gt_{l}")
                for g in range(ncall):
                    gi = nc.gpsimd.indirect_dma_start(
                        out=gt[:, g, :], out_offset=None, in_=store[:],
                        in_offset=bass.IndirectOffsetOnAxis(
                            ap=idxt[:, g0 + g:g0 + g + 1], axis=0))
                    add_dep_helper(gi.ins, prev_ag.ins, reason=f"gather l{l}")

                xt = [pool.tile([P, XTW], bf16, tag=f"xt{m}", name=f"xt{m}_{l}")
                      for m in range(M)]
                for m in range(M):
                    for blk in range((ncall + 3) // 4):
                        n_s = min(4, ncall - blk * 4)
                        ps = tp.tile([P, 512], bf16, tag="tpsb", name=f"tps_{l}_{m}_{blk}")
                        for k in range(n_s):
                            s_ = blk * 4 + k
                            nc.tensor.transpose(out=ps[:, k * P:(k + 1) * P],
                                                in_=gt[:, s_, m * D:(m + 1) * D],
                                                identity=ident[:])
                        if (m + blk) % 2:
                            nc.scalar.copy(out=xt[m][:, blk * 512: blk * 512 + n_s * P],
                                           in_=ps[:, 0:n_s * P])
                        else:
                            nc.vector.tensor_copy(out=xt[m][:, blk * 512: blk * 512 + n_s * P],
                                                  in_=ps[:, 0:n_s * P])

                hrelu = [pool.tile([P, 1152], bf16, tag=f"hr{m}", name=f"hr{m}_{l}")
                         for m in range(M)]
                for m in range(M):
                    nc.vector.memset(hrelu[m][:], 0.0)
                    for half in range(2):
                        r0, r1 = (0, 4) if half == 0 else (4, 8)
                        c0 = int(gof[l, r0])
                        width = int(gof[l, r1]) - c0
                        if width == 0:
                            continue
                        hps = hp.tile([P, 512], f32, tag="hps", name=f"hps_{l}_{m}_{half}")
                        for r in range(r0, r1):
                            a_r, b_r = int(gof[l, r]), int(gof[l, r + 1])
                            if b_r == a_r:
                                continue
                            w1i = (m * R + r) * 2
                            nc.tensor.matmul(out=hps[:, a_r - c0:b_r - c0],
                                             lhsT=w1sb[:, w1i, :],
                                             rhs=xt[m][:, a_r:b_r], start=True, stop=False)
                            nc.tensor.matmul(out=hps[:, a_r - c0:b_r - c0],
                                             lhsT=w1sb[:, w1i + 1, :],
                                             rhs=xt[m][:, Sl + a_r:Sl + b_r],
                                             start=False, stop=True)
                        if zero_b1:
                            nc.scalar.activation(out=hrelu[m][:, c0:c0 + width],
                                                 in_=hps[:, 0:width], func=AF.Relu)
                        else:
                            for r in range(r0, r1):
                                a_r, b_r = int(gof[l, r]), int(gof[l, r + 1])
                                if b_r == a_r:
                                    continue
                                nc.scalar.activation(out=hrelu[m][:, a_r:b_r],
                                                     in_=hps[:, a_r - c0:b_r - c0],
                                                     func=AF.Relu,
                                                     bias=b1t[:, m * R + r: m * R + r + 1])

                slab = pool.tile([P, R, 2 * D], bf16, tag="slab", name=f"slab_{l}")
                for m in range(M):
                    for bank in range(3):
                        rr = list(range(bank * 3, min(R, bank * 3 + 3)))
                        l2 = l2p.tile([P, 512], f32, tag="l2ps", name=f"l2_{l}_{m}_{bank}")
                        for j, r in enumerate(rr):
                            a_r = int(gof[l, r])
                            nc.tensor.matmul(out=l2[:, j * 129:(j + 1) * 129],
                                             lhsT=hrelu[m][:, a_r:a_r + P],
                                             rhs=bp[:, m * R + r, :], start=True, stop=True)
                        if not zero_b2:
                            for j, r in enumerate(rr):
                                nc.vector.tensor_tensor(
                                    out=l2[:, j * 129:j * 129 + D],
                                    in0=l2[:, j * 129:j * 129 + D],
                                    in1=b2bc[:, m * R + r, :], op=ALU.add)
                        nc.vector.tensor_copy(
                            out=valsm[m][:, IBLK + l * R + bank * 3:
                                         IBLK + l * R + bank * 3 + len(rr)],
                            in_=l2[:, D:D + 1 + (len(rr) - 1) * 129:129])
                        for j, r in enumerate(rr):
                            if (m + j) % 2:
                                nc.scalar.copy(out=slab[:, r, m * D:(m + 1) * D],
                                               in_=l2[:, j * 129:j * 129 + D])
                            else:
                                nc.vector.tensor_copy(out=slab[:, r, m * D:(m + 1) * D],
                                                      in_=l2[:, j * 129:j * 129 + D])

                base = int(NB[l])
                sbounce = dpool.tile([R * P, 2 * D], bf16, tag="sbounce", name=f"sb_{l}")
                nc.sync.dma_start(out=sbounce[:].rearrange("(a p) r -> p a r", p=P),
                                  in_=slab[:])
                prev_ag = nc.gpsimd.collective_compute(
                    "AllGather", ALU.bypass, replica_groups=[list(range(NC_))],
                    ins=[sbounce[:]], outs=[store[base: base + NC_ * P * R]])
                tc.dep_state.clear_tensor_accesses(store.tensor.name)

            # ---------------- BCE tail ----------------
            acc = persist.tile([P, 6], f32)
            for m in range(M):
                v = valsm[m]
                if b_eval_vals[m] != 0.0:
                    nc.vector.tensor_scalar(out=v[:], in0=v[:], scalar1=float(b_eval_vals[m]),
                                            scalar2=None, op0=ALU.add)
                sp = pool.tile([P, VCOLS], f32, tag="sp")
                nc.scalar.activation(out=sp[:], in_=v[:], func=AF.Abs)
                nc.scalar.activation(out=sp[:], in_=sp[:], func=AF.Exp, scale=-1.0)
                nc.scalar.activation(out=sp[:], in_=sp[:], func=AF.Ln, bias=1.0)
                rl = pool.tile([P, VCOLS], f32, tag="rl")
                nc.scalar.activation(out=rl[:], in_=v[:], func=AF.Relu)
                nc.vector.tensor_tensor(out=sp[:], in0=sp[:], in1=rl[:], op=ALU.add)
                t = pool.tile([P, VCOLS], f32, tag="bce")
                nc.vector.tensor_tensor(out=t[:], in0=sp[:], in1=cot[:, 0, :], op=ALU.mult)
                t2 = pool.tile([P, VCOLS], f32, tag="bce2")
                nc.vector.tensor_tensor(out=t2[:], in0=v[:], in1=cot[:, 1, :], op=ALU.mult)
                nc.vector.tensor_tensor(out=t[:], in0=t[:], in1=t2[:], op=ALU.subtract)
                nc.vector.reduce_sum(out=acc[:, m:m + 1], in_=t[:], axis=AX.X)
                ge = pool.tile([P, VCOLS], f32, tag="ge")
                nc.vector.tensor_scalar(out=ge[:], in0=v[:], scalar1=0.0, scalar2=None,
                                        op0=ALU.is_ge)
                nc.vector.tensor_tensor(out=t[:], in0=ge[:], in1=cot[:, 2, :], op=ALU.mult)
                nc.vector.reduce_sum(out=acc[:, 2 + m:3 + m], in_=t[:], axis=AX.X)
                nc.vector.tensor_tensor(out=t[:], in0=ge[:], in1=cot[:, 3, :], op=ALU.mult)
                nc.vector.reduce_sum(out=acc[:, 4 + m:5 + m], in_=t[:], axis=AX.X)
            rps = tp.tile([P, 512], f32, tag="tps")
            nc.tensor.matmul(out=rps[0:6, 0:1], lhsT=acc[:], rhs=ones_col[:],
                             start=True, stop=True)
            outt = pool.tile([6, 1], f32, tag="outt")
            nc.vector.tensor_copy(out=outt[:], in_=rps[0:6, 0:1])
            nc.sync.dma_start(out=out[0:6, None], in_=outt[:])

    nc.compile()
    return nc


def kernel(thax_ids, sine_ids, parents, rule_ids, pos_cnt, neg_cnt,
           thax_table, sine_w, sine_b, W1, b1, W2, b2, w_eval, b_eval):
    prep = _host_prep(thax_ids, sine_ids, parents, rule_ids, pos_cnt, neg_cnt)
    zero_b1 = not np.any(np.asarray(b1))
    zero_b2 = not np.any(np.asarray(b2))
    b_eval_vals = [float(x) for x in np.asarray(b_eval)]
    nc = _build(prep, zero_b1, zero_b2, b_eval_vals)

    common = dict(
        thax0=np.ascontiguousarray(np.asarray(thax_table, np.float32)[0]),
        thax1=np.ascontiguousarray(np.asarray(thax_table, np.float32)[1]),
        W1=np.ascontiguousarray(np.asarray(W1, np.float32)),
        W2=np.ascontiguousarray(np.asarray(W2, np.float32)),
        b1=np.ascontiguousarray(np.asarray(b1, np.float32)),
        b2=np.ascontiguousarray(np.asarray(b2, np.float32)),
        sine_w=np.ascontiguousarray(np.asarray(sine_w, np.float32)),
        sine_b=np.ascontiguousarray(np.asarray(sine_b, np.float32)),
        w_eval=np.ascontiguousarray(np.asarray(w_eval, np.float32)),
    )
    in_maps = []
    for c in range(NC_):
        in_maps.append(dict(common,
                            gidx=prep["gidx"][c], tidx=prep["tidx"][c],
                            sval=prep["sval"][c], co=prep["co"][c]))
    trace = os.environ.get("KTRACE", "0") == "1"
    res = run_bass_kernel_spmd(nc, in_maps, core_ids=list(range(NC_)), trace=trace)
    if trace and res.exec_time_ns is not None:
        print(f"HW exec time: {res.exec_time_ns} ns")

    loss = np.zeros(M, np.float64)
    posOK = np.zeros(M, np.float64)
    negge = np.zeros(M, np.float64)
    for c in range(NC_):
        o = np.asarray(res.results[c]["out"], np.float64)
        loss += o[0:2]
        posOK += o[2:4]
        negge += o[4:6]
    negOK = prep["mneg_total"] - negge
    return np.stack([loss, posOK, negOK]).astype(np.float32)


# revision 19
# speedup vs baseline: 1.3478x; 1.2596x over previous
"""Trainium2 Bass kernel for nn_EvalMultiModel (gnn_message_passing, 8 cores).

Sharding: derivation-node axis K split 8 ways (512 nodes/level/core), both
ensemble members on every core.  The embedding store holds rows [m0|m1]
(256 bf16 = 512 B) in Shared DRAM; each level's new rows are exchanged with an
8-rank AllGather that writes straight into the store slice for that level.
Parent rows are fetched with indirect DMA (128 rows/call, int32 row indices,
compile-time-specialized on the `parents` input), transposed on the tensor
engine to feature-major, then per rule:
    L1: h^T = W1_r^T x^T   (W1 stationary bf16, fp32 PSUM), fused relu
    L2: streams [W2_r | W2_r w_eval] against relu(h)^T as stationary, so the
        new embedding comes out node-major with the eval logit as a free
        129th column.
The weighted-BCE / posOK / negOK reductions run on device per core; the host
sums the eight partial results.  Host-side python only prepares integer
index/schedule tables and the loss coefficient vectors.
"""
import os
import sys

if "/opt/trn_rl_repo" not in sys.path:
    sys.path.insert(0, "/opt/trn_rl_repo")

import numpy as np

import concourse.bass as bass
import concourse.bacc as bacc
import concourse.mybir as mybir
import concourse.tile as tile
from concourse.bass_utils import run_bass_kernel_spmd
from concourse.masks import make_identity
from bass_rust import add_dep_helper

M, D, R = 2, 128, 8
N0, L, K = 8192, 32, 4096
POS_WEIGHT_EXTRA = 1.0
NC_ = 8
KC = K // NC_              # 512 real nodes per core per level
P = 128
INITC = N0 // NC_          # 1024 init nodes per core
IBLK = INITC // P          # 8
GCALLS = 9                 # indirect gather calls per level (1152 padded rows)
VCOLS = IBLK + L * R       # 264 vals columns per m

f32 = mybir.dt.float32
bf16 = mybir.dt.bfloat16
i32 = mybir.dt.int32
AF = mybir.ActivationFunctionType
ALU = mybir.AluOpType
AX = mybir.AxisListType


def _host_prep(thax_ids, sine_ids, parents, rule_ids, pos_cnt, neg_cnt):
    thax_ids = np.asarray(thax_ids); sine_ids = np.asarray(sine_ids)
    parents = np.asarray(parents); rule_ids = np.asarray(rule_ids)
    pos_cnt = np.asarray(pos_cnt, np.float64); neg_cnt = np.asarray(neg_cnt, np.float64)

    u = np.zeros((L, R), np.int64)
    for l in range(L):
        cnt = np.bincount(rule_ids[l], minlength=R)
        u[l] = -(-cnt // NC_)

    # real assignment: per (level, rule) deal round-robin
    corec = np.zeros((L, K), np.int64)
    coreq = np.zeros((L, K), np.int64)
    reals = [[None] * NC_ for _ in range(L)]   # per (l, c): [R][...] node offsets
    for l in range(L):
        order = np.argsort(rule_ids[l], kind="stable")
        rs = rule_ids[l][order]
        starts = np.searchsorted(rs, np.arange(R + 1))
        for c in range(NC_):
            reals[l][c] = [order[starts[r]:starts[r + 1]][c::NC_] for r in range(R)]
        for r in range(R):
            ids = order[starts[r]:starts[r + 1]]
            corec[l, ids] = np.arange(len(ids)) % NC_
            coreq[l, ids] = np.arange(len(ids)) // NC_

    LEV = NC_ * P * R          # padded real rows per level
    DUPR = P * R               # dup-region rows per level
    NB = np.zeros(L + 1, np.int64)
    NB[0] = N0
    for l in range(L):
        NB[l + 1] = NB[l] + LEV + DUPR
    NSTORE = int(NB[L])

    def real_row(nids):
        nids = np.asarray(nids, np.int64)
        lev = np.clip((nids - N0) // K, 0, L - 1)
        off = (nids - N0) % K
        rr = rule_ids[lev, off]
        pos = NB[lev] + corec[lev, off] * (P * R) + rr * P + coreq[lev, off]
        return np.where(nids < N0, np.maximum(nids, 0), pos)

    # backward closure of dup sets: dup_nodes[lc][c] = level-lc nodes, not owned
    # by c, referenced by core c's batch at level lc+1 (reals + its dups).
    USE_DUPS = True
    dup_nodes = [[np.zeros(0, np.int64) for _ in range(NC_)] for _ in range(L)]
    if USE_DUPS:
        # pair-union, capped per rule at 128 - u; symmetric within HBM-sharing
        # pairs {2k, 2k+1} so concurrent identical writes are benign.
        for l in range(L - 1, 0, -1):
            for pair in range(NC_ // 2):
                c0, c1 = 2 * pair, 2 * pair + 1
                refs = []
                for c in (c0, c1):
                    roff = np.concatenate(reals[l][c]).astype(np.int64)
                    doff = ((dup_nodes[l][c] - N0) % K).astype(np.int64)
                    batch = np.concatenate([roff, doff])
                    par = parents[l, batch].reshape(-1).astype(np.int64)
                    lev = (par - N0) // K
                    refs.append(par[(par >= N0) & (lev == l - 1)])
                tgt = np.unique(np.concatenate(refs)) if refs else np.zeros(0, np.int64)
                # cap per rule
                if len(tgt):
                    rr = rule_ids[l - 1, (tgt - N0) % K]
                    keep = []
                    for r in range(R):
                        cap = P - int(u[l - 1, r])
                        sel = np.nonzero(rr == r)[0]
                        keep.append(tgt[sel[:cap]])
                    tgt = np.concatenate(keep)
                dup_nodes[l - 1][c0] = tgt
                dup_nodes[l - 1][c1] = tgt

    d = np.zeros((L, R), np.int64)
    dq = [[None] * NC_ for _ in range(L)]
    for lc in range(L):
        dcnt = np.zeros((NC_, R), np.int64)
        for c in range(NC_):
            tg = dup_nodes[lc][c]
            if len(tg):
                dcnt[c] = np.bincount(rule_ids[lc, (tg - N0) % K], minlength=R)
        d[lc] = dcnt.max(0)
        assert (u[lc] + d[lc]).max() <= P, f"rule block overflow lvl {lc}"
        for c in range(NC_):
            tg = dup_nodes[lc][c]
            mp = {}
            if len(tg):
                rr = rule_ids[lc, (tg - N0) % K]
                o = np.argsort(rr, kind="stable")
                pos = np.zeros(R, np.int64)
                for nid, r in zip(tg[o], rr[o]):
                    mp[int(nid)] = (int(r), int(pos[r]))
                    pos[r] += 1
            dq[lc][c] = mp

    gof = np.zeros((L, R + 1), np.int64)
    S = np.zeros(L, np.int64)
    gcalls = np.zeros(L, np.int64)
    for l in range(L):
        gof[l, 1:] = np.cumsum(u[l] + d[l])
        S[l] = gof[l, R]
        gcalls[l] = -(-2 * S[l] // P)
    goff = np.zeros(L + 1, np.int64)
    for l in range(L):
        goff[l + 1] = goff[l] + gcalls[l]
    TOTG = int(goff[L])

    def ref_row(par, l, c):
        """returns (rows, fresh_mask): fresh = still depends on AG(l-1)."""
        par = np.asarray(par, np.int64)
        out = real_row(par)
        fresh = np.zeros(len(par), bool)
        lev = (par - N0) // K
        sel = (par >= N0) & (lev == l - 1)
        for i in np.nonzero(sel)[0]:
            nid = int(par[i])
            hit = dq[l - 1][c].get(nid) if USE_DUPS else None
            if hit is not None:
                r, j = hit
                out[i] = NB[l - 1] + LEV + r * P + int(u[l - 1, r]) + j
            else:
                fresh[i] = True
        return out, fresh

    # slot lists, gather indices
    gidx = np.zeros((NC_, P, TOTG), np.int32)
    callfresh = np.zeros(TOTG, bool)
    Smax = int(S.max())
    slotnode = np.full((L, NC_, Smax), -1, np.int64)
    for l in range(L):
        Sl = int(S[l])
        for c in range(NC_):
            for r in range(R):
                base = int(gof[l, r])
                ids = reals[l][c][r]
                for q, nid in enumerate(ids):
                    slotnode[l, c, base + q] = N0 + l * K + nid
            for nid, (r, j) in dq[l][c].items():
                slotnode[l, c, int(gof[l, r]) + int(u[l, r]) + j] = nid
            sn = slotnode[l, c, :Sl]
            ok = sn >= 0
            off = np.where(ok, (sn - N0) % K, 0)
            pn = parents[l, off].astype(np.int64)
            pn[~ok] = 0
            lst = np.zeros(int(gcalls[l]) * P, np.int64)
            fr = np.zeros(int(gcalls[l]) * P, bool)
            r0_, f0_ = ref_row(pn[:, 0], l, c)
            r1_, f1_ = ref_row(pn[:, 1], l, c)
            lst[:Sl] = np.where(ok, r0_, 0)
            lst[Sl:2 * Sl] = np.where(ok, r1_, 0)
            fr[:Sl] = ok & f0_
            fr[Sl:2 * Sl] = ok & f1_
            gidx[c, :, int(goff[l]):int(goff[l + 1])] = \
                lst.reshape(int(gcalls[l]), P).T.astype(np.int32)
            callfresh[int(goff[l]):int(goff[l + 1])] |= \
                fr.reshape(int(gcalls[l]), P).any(1)

    tidx = np.zeros((NC_, P, IBLK), np.int32)
    sval = np.zeros((NC_, P, IBLK), np.float32)
    for c in range(NC_):
        mine = np.arange(c * INITC, (c + 1) * INITC)
        tidx[c] = thax_ids[mine].reshape(IBLK, P).T
        sval[c] = sine_ids[mine].astype(np.float32).reshape(IBLK, P).T

    cnt = pos_cnt + neg_cnt
    mask = (cnt > 0).astype(np.float64)
    gold = np.where(cnt > 0, pos_cnt / np.maximum(cnt, 1.0), 0.0)
    tp_, tn_ = pos_cnt.sum(), neg_cnt.sum()
    pw = POS_WEIGHT_EXTRA * tn_ / max(tp_, 1.0) if tp_ > 0 else 1.0
    a = pw * gold * mask * cnt
    ab = a + (1.0 - gold) * mask * cnt
    mpos = mask * pos_cnt
    mneg = mask * neg_cnt

    co = np.zeros((NC_, 4, P, VCOLS), np.float32)
    for c in range(NC_):
        mine = np.arange(c * INITC, (c + 1) * INITC)
        for j, arr in enumerate((ab, a, mpos, mneg)):
            co[c, j, :, :IBLK] = arr[mine].reshape(IBLK, P).T
        for l in range(L):
            for r in range(R):
                ids = reals[l][c][r]
                nid = N0 + l * K + ids
                for j, arr in enumerate((ab, a, mpos, mneg)):
                    co[c, j, 0:len(ids), IBLK + l * R + r] = arr[nid]
    return dict(S=S, gof=gof, u=u, NB=NB, NSTORE=NSTORE, gidx=gidx, tidx=tidx,
                sval=sval, co=co, mneg_total=float(mneg.sum()),
                gcalls=gcalls, goff=goff, TOTG=TOTG, LEV=LEV,
                d=(gof[:, 1:] - gof[:, :-1]) - u, callfresh=callfresh)


def _build(prep, zero_b1, zero_b2, b_eval_vals):
    S, gof, NB, NSTORE = prep["S"], prep["gof"], prep["NB"], prep["NSTORE"]
    nc = bacc.Bacc("TRN2", target_bir_lowering=False, debug=False, num_devices=NC_)

    def dt_in(n, s, d=f32):
        return nc.dram_tensor(n, s, d, kind="ExternalInput").ap()

    TOTG = prep["TOTG"]
    gcalls, goff = prep["gcalls"], prep["goff"]
    gidx = dt_in("gidx", [P, TOTG], i32)
    tidx = dt_in("tidx", [P, IBLK], i32)
    sval = dt_in("sval", [P, IBLK])
    co = dt_in("co", [4, P, VCOLS])
    thax0 = dt_in("thax0", [1000, D])
    thax1 = dt_in("thax1", [1000, D])
    W1 = dt_in("W1", [M, R, 2 * D, D])
    W2 = dt_in("W2", [M, R, D, D])
    b1 = dt_in("b1", [M, R, D])
    b2 = dt_in("b2", [M, R, D])
    sine_w = dt_in("sine_w", [M, D])
    sine_b = dt_in("sine_b", [M, D])
    w_eval = dt_in("w_eval", [M, D])
    out = nc.dram_tensor("out", [8], f32, kind="ExternalOutput").ap()
    store = nc.dram_tensor("store", [NSTORE, 2 * D], bf16, addr_space="Shared").ap()

    with tile.TileContext(nc) as tc:
        with (
            tc.tile_pool(name="persist", bufs=1) as persist,
            tc.tile_pool(name="initp", bufs=1) as initp,
            tc.tile_pool(name="work", bufs=2) as pool,
            tc.tile_pool(name="gt", bufs=3) as gpool,
            tc.tile_pool(name="tp", bufs=2, space="PSUM") as tp,
            tc.tile_pool(name="hp", bufs=2, space="PSUM") as hp,
            tc.tile_pool(name="l2p", bufs=2, space="PSUM") as l2p,
            tc.tile_pool(name="dram", bufs=2, space="DRAM") as dpool,
        ):
            # ---------------- one-time prep ----------------
            ident = persist.tile([P, P], bf16)
            make_identity(nc, ident[:])
            onesc = persist.tile([1, P], f32)
            nc.vector.memset(onesc[:], 1.0)
            ones_col = persist.tile([P, 1], f32)
            nc.vector.memset(ones_col[:], 1.0)

            idxt = persist.tile([P, TOTG], i32)
            nc.sync.dma_start(out=idxt[:], in_=gidx[:])
            tidxt = persist.tile([P, IBLK], i32)
            nc.sync.dma_start(out=tidxt[:], in_=tidx[:])
            cot = persist.tile([P, 4, VCOLS], f32)
            nc.sync.dma_start(out=cot[:], in_=co.rearrange("j p v -> p j v"))

            w1f = initp.tile([P, M * R * 2, D], f32)
            nc.sync.dma_start(out=w1f[:], in_=W1.rearrange("m r (t p) e -> p (m r t) e", p=P))
            w1sb = persist.tile([P, M * R * 2, D], bf16)
            nc.vector.tensor_copy(out=w1sb[:], in_=w1f[:])

            w2f = initp.tile([P, M * R, D], f32)
            nc.sync.dma_start(out=w2f[:], in_=W2.rearrange("m r e f -> e (m r) f"))

            b1t = persist.tile([P, M * R], f32)
            nc.sync.dma_start(out=b1t[:], in_=b1.rearrange("m r d -> d (m r)"))

            brows = initp.tile([1, 6, D], f32)
            nc.sync.dma_start(out=brows[:, 0:2, :], in_=sine_w.rearrange("m d -> m d")[None])
            nc.sync.dma_start(out=brows[:, 2:4, :], in_=sine_b[None])
            nc.sync.dma_start(out=brows[:, 4:6, :], in_=w_eval[None])
            bc = persist.tile([P, 6, D], f32)  # w_m0 w_m1 b_m0 b_m1 ev_m0 ev_m1
            for j in range(6):
                bps = tp.tile([P, 512], f32, tag="tps")
                nc.tensor.matmul(out=bps[:, 0:P], lhsT=onesc[:], rhs=brows[:, j, :],
                                 start=True, stop=True)
                nc.vector.tensor_copy(out=bc[:, j, :], in_=bps[:, 0:P])

            # b2 broadcast rows (only if nonzero): [P, (m r), D]
            if not zero_b2:
                b2r = initp.tile([1, M * R, D], f32)
                nc.sync.dma_start(out=b2r[:], in_=b2.rearrange("m r d -> (m r) d")[None])
                b2bc = persist.tile([P, M * R, D], f32)
                for j in range(M * R):
                    bps = tp.tile([P, 512], f32, tag="tps")
                    nc.tensor.matmul(out=bps[:, 0:P], lhsT=onesc[:], rhs=b2r[:, j, :],
                                     start=True, stop=True)
                    nc.vector.tensor_copy(out=b2bc[:, j, :], in_=bps[:, 0:P])

            # B' = [W2_mr (+b2 fold NO) | W2_mr @ w_eval_m] bf16
            bp = persist.tile([P, M * R, D + 1], bf16)
            for m in range(M):
                for r in range(R):
                    i = m * R + r
                    t = initp.tile([P, D], f32, tag="w2e")
                    nc.vector.tensor_tensor(out=t[:], in0=w2f[:, i, :], in1=bc[:, 4 + m, :], op=ALU.mult)
                    t2 = initp.tile([P, 1], f32, tag="w2e2")
                    nc.vector.reduce_sum(out=t2[:], in_=t[:], axis=AX.X)
                    nc.scalar.copy(out=bp[:, i, 0:D], in_=w2f[:, i, :])
                    nc.vector.tensor_copy(out=bp[:, i, D:D + 1], in_=t2[:])

            # ---------------- init nodes ----------------
            vals0 = persist.tile([P, VCOLS], f32)
            vals1 = persist.tile([P, VCOLS], f32)
            valsm = [vals0, vals1]
            svt = persist.tile([P, IBLK], f32)
            nc.sync.dma_start(out=svt[:], in_=sval[:])
            islab = initp.tile([P, IBLK, 2 * D], bf16)
            for m in range(M):
                e = initp.tile([P, IBLK, D], f32, tag=f"emb{m}")
                th = (thax0, thax1)[m]
                for g in range(IBLK):
                    nc.gpsimd.indirect_dma_start(
                        out=e[:, g, :], out_offset=None, in_=th[:],
                        in_offset=bass.IndirectOffsetOnAxis(ap=tidxt[:, g:g + 1], axis=0))
                for g in range(IBLK):
                    t = initp.tile([P, D], f32, tag="sig")
                    nc.vector.tensor_tensor(out=t[:], in0=svt[:, g:g + 1].to_broadcast([P, D]),
                                            in1=bc[:, m, :], op=ALU.mult)
                    nc.vector.tensor_tensor(out=t[:], in0=t[:], in1=bc[:, 2 + m, :], op=ALU.add)
                    nc.scalar.activation(out=t[:], in_=t[:], func=AF.Sigmoid)
                    nc.vector.tensor_tensor(out=e[:, g, :], in0=e[:, g, :], in1=t[:], op=ALU.mult)
                    t2 = initp.tile([P, D], f32, tag="sig2")
                    nc.vector.tensor_tensor(out=t2[:], in0=e[:, g, :], in1=bc[:, 4 + m, :], op=ALU.mult)
                    nc.vector.reduce_sum(out=valsm[m][:, g:g + 1], in_=t2[:], axis=AX.X)
                nc.scalar.copy(out=islab[:, :, m * D:(m + 1) * D], in_=e[:])
            ibounce = dpool.tile([INITC, 2 * D], bf16, tag="ibounce")
            nc.sync.dma_start(out=ibounce[:].rearrange("(a p) r -> p a r", p=P), in_=islab[:])
            prev_ag = nc.gpsimd.collective_compute(
                "AllGather", ALU.bypass, replica_groups=[list(range(NC_))],
                ins=[ibounce[:]], outs=[store[0:N0]])
            tc.dep_state.clear_tensor_accesses(store.tensor.name)

            # ---------------- levels ----------------
            XTW = int(max(gcalls)) * P
            uarr, darr, callfresh = prep["u"], prep["d"], prep["callfresh"]
            ag2 = prev_ag      # AG(l-2) stand-in
            dupw_prev = None
            for l in range(L):
                Sl = int(S[l])
                ncall = int(gcalls[l])
                g0 = int(goff[l])
                gt = gpool.tile([P, ncall, 2 * D], bf16, tag="gt", name=f"gt_{l}")
                for g in range(ncall):
                    gi = nc.gpsimd.indirect_dma_start(
                        out=gt[:, g, :], out_offset=None, in_=store[:],
                        in_offset=bass.IndirectOffsetOnAxis(
                            ap=idxt[:, g0 + g:g0 + g + 1], axis=0))
                    dep = prev_ag if callfresh[g0 + g] else ag2
                    add_dep_helper(gi.ins, dep.ins, reason=f"gather l{l} c{g}")
                    if dupw_prev is not None:
                        add_dep_helper(gi.ins, dupw_prev.ins, reason=f"gather l{l} dup")

                xt = [pool.tile([P, XTW], bf16, tag=f"xt{m}", name=f"xt{m}_{l}")
                      for m in range(M)]
                for m in range(M):
                    for blk in range((ncall + 3) // 4):
                        n_s = min(4, ncall - blk * 4)
                        ps = tp.tile([P, 512], bf16, tag="tpsb", name=f"tps_{l}_{m}_{blk}")
                        for k in range(n_s):
                            s_ = blk * 4 + k
                            nc.tensor.transpose(out=ps[:, k * P:(k + 1) * P],
                                                in_=gt[:, s_, m * D:(m + 1) * D],
                                                identity=ident[:])
                        if (m + blk) % 2:
                            nc.scalar.copy(out=xt[m][:, blk * 512: blk * 512 + n_s * P],
                                           in_=ps[:, 0:n_s * P])
                        else:
                            nc.vector.tensor_copy(out=xt[m][:, blk * 512: blk * 512 + n_s * P],
                                                  in_=ps[:, 0:n_s * P])

                hrelu = [pool.tile([P, 1152], bf16, tag=f"hr{m}", name=f"hr{m}_{l}")
                         for m in range(M)]
                for m in range(M):
                    nc.vector.memset(hrelu[m][:], 0.0)
                    for half in range(2):
                        r0, r1 = (0, 4) if half == 0 else (4, 8)
                        c0 = int(gof[l, r0])
                        width = int(gof[l, r1]) - c0
                        if width == 0:
                            continue
                        hps = hp.tile([P, 512], f32, tag="hps", name=f"hps_{l}_{m}_{half}")
                        for r in range(r0, r1):
                            a_r, b_r = int(gof[l, r]), int(gof[l, r + 1])
                            if b_r == a_r:
                                continue
                            w1i = (m * R + r) * 2
                            nc.tensor.matmul(out=hps[:, a_r - c0:b_r - c0],
                                             lhsT=w1sb[:, w1i, :],
                                             rhs=xt[m][:, a_r:b_r], start=True, stop=False)
                            nc.tensor.matmul(out=hps[:, a_r - c0:b_r - c0],
                                             lhsT=w1sb[:, w1i + 1, :],
                                             rhs=xt[m][:, Sl + a_r:Sl + b_r],
                                             start=False, stop=True)
                        if zero_b1:
                            nc.scalar.activation(out=hrelu[m][:, c0:c0 + width],
                                                 in_=hps[:, 0:width], func=AF.Relu)
                        else:
                            for r in range(r0, r1):
                                a_r, b_r = int(gof[l, r]), int(gof[l, r + 1])
                                if b_r == a_r:
                                    continue
                                nc.scalar.activation(out=hrelu[m][:, a_r:b_r],
                                                     in_=hps[:, a_r - c0:b_r - c0],
                                                     func=AF.Relu,
                                                     bias=b1t[:, m * R + r: m * R + r + 1])

                slab = pool.tile([P, R, 2 * D], bf16, tag="slab", name=f"slab_{l}")
                dupslab = pool.tile([P, R, 2 * D], bf16, tag="dupslab", name=f"dupslab_{l}")
                for m in range(M):
                    for bank in range(3):
                        rr = list(range(bank * 3, min(R, bank * 3 + 3)))
                        l2 = l2p.tile([P, 512], f32, tag="l2ps", name=f"l2_{l}_{m}_{bank}")
                        for j, r in enumerate(rr):
                            a_r = int(gof[l, r])
                            nc.tensor.matmul(out=l2[:, j * 129:(j + 1) * 129],
                                             lhsT=hrelu[m][:, a_r:a_r + P],
                                             rhs=bp[:, m * R + r, :], start=True, stop=True)
                        if not zero_b2:
                            for j, r in enumerate(rr):
                                nc.vector.tensor_tensor(
                                    out=l2[:, j * 129:j * 129 + D],
                                    in0=l2[:, j * 129:j * 129 + D],
                                    in1=b2bc[:, m * R + r, :], op=ALU.add)
                        nc.vector.tensor_copy(
                            out=valsm[m][:, IBLK + l * R + bank * 3:
                                         IBLK + l * R + bank * 3 + len(rr)],
                            in_=l2[:, D:D + 1 + (len(rr) - 1) * 129:129])
                        for j, r in enumerate(rr):
                            if (m + j) % 2:
                                nc.scalar.copy(out=slab[:, r, m * D:(m + 1) * D],
                                               in_=l2[:, j * 129:j * 129 + D])
                            else:
                                nc.vector.tensor_copy(out=slab[:, r, m * D:(m + 1) * D],
                                                      in_=l2[:, j * 129:j * 129 + D])
                            u_r, d_r = int(uarr[l, r]), int(darr[l, r])
                            if d_r > 0:
                                lo = 0
                                hi = P
                                if (m + j) % 2 == 0:
                                    nc.scalar.copy(
                                        out=dupslab[lo:hi, r, m * D:(m + 1) * D],
                                        in_=l2[lo:hi, j * 129:j * 129 + D])
                                else:
                                    nc.vector.tensor_copy(
                                        out=dupslab[lo:hi, r, m * D:(m + 1) * D],
                                        in_=l2[lo:hi, j * 129:j * 129 + D])

                base = int(NB[l])
                sbounce = dpool.tile([R * P, 2 * D], bf16, tag="sbounce", name=f"sb_{l}")
                nc.sync.dma_start(out=sbounce[:].rearrange("(a p) r -> p a r", p=P),
                                  in_=slab[:])
                if int(darr[l].sum()) > 0:
                    dupw_prev = nc.sync.dma_start(
                        out=store[base + NC_ * P * R: base + NC_ * P * R + P * R]
                            .rearrange("(a p) r -> p a r", p=P),
                        in_=dupslab[:])
                else:
                    dupw_prev = None
                ag2 = prev_ag
                prev_ag = nc.gpsimd.collective_compute(
                    "AllGather", ALU.bypass, replica_groups=[list(range(NC_))],
                    ins=[sbounce[:]], outs=[store[base: base + NC_ * P * R]])
                tc.dep_state.clear_tensor_accesses(store.tensor.name)

            # ---------------- BCE tail ----------------
            acc = persist.tile([P, 6], f32)
            for m in range(M):
                v = valsm[m]
                if b_eval_vals[m] != 0.0:
                    nc.vector.tensor_scalar(out=v[:], in0=v[:], scalar1=float(b_eval_vals[m]),
                                            scalar2=None, op0=ALU.add)
                sp = pool.tile([P, VCOLS], f32, tag="sp")
                nc.scalar.activation(out=sp[:], in_=v[:], func=AF.Abs)
                nc.scalar.activation(out=sp[:], in_=sp[:], func=AF.Exp, scale=-1.0)
                nc.scalar.activation(out=sp[:], in_=sp[:], func=AF.Ln, bias=1.0)
                rl = pool.tile([P, VCOLS], f32, tag="rl")
                nc.scalar.activation(out=rl[:], in_=v[:], func=AF.Relu)
                nc.vector.tensor_tensor(out=sp[:], in0=sp[:], in1=rl[:], op=ALU.add)
                t = pool.tile([P, VCOLS], f32, tag="bce")
                nc.vector.tensor_tensor(out=t[:], in0=sp[:], in1=cot[:, 0, :], op=ALU.mult)
                t2 = pool.tile([P, VCOLS], f32, tag="bce2")
                nc.vector.tensor_tensor(out=t2[:], in0=v[:], in1=cot[:, 1, :], op=ALU.mult)
                nc.vector.tensor_tensor(out=t[:], in0=t[:], in1=t2[:], op=ALU.subtract)
                nc.vector.reduce_sum(out=acc[:, m:m + 1], in_=t[:], axis=AX.X)
                ge = pool.tile([P, VCOLS], f32, tag="ge")
                nc.vector.tensor_scalar(out=ge[:], in0=v[:], scalar1=0.0, scalar2=None,
                                        op0=ALU.is_ge)
                nc.vector.tensor_tensor(out=t[:], in0=ge[:], in1=cot[:, 2, :], op=ALU.mult)
                nc.vector.reduce_sum(out=acc[:, 2 + m:3 + m], in_=t[:], axis=AX.X)
                nc.vector.tensor_tensor(out=t[:], in0=ge[:], in1=cot[:, 3, :], op=ALU.mult)
                nc.vector.reduce_sum(out=acc[:, 4 + m:5 + m], in_=t[:], axis=AX.X)
            rps = tp.tile([P, 512], f32, tag="tps")
            nc.tensor.matmul(out=rps[0:6, 0:1], lhsT=acc[:], rhs=ones_col[:],
                             start=True, stop=True)
            outt = pool.tile([6, 1], f32, tag="outt")
            nc.vector.tensor_copy(out=outt[:], in_=rps[0:6, 0:1])
            nc.sync.dma_start(out=out[0:6, None], in_=outt[:])

    nc.compile()
    return nc


def kernel(thax_ids, sine_ids, parents, rule_ids, pos_cnt, neg_cnt,
           thax_table, sine_w, sine_b, W1, b1, W2, b2, w_eval, b_eval):
    prep = _host_prep(thax_ids, sine_ids, parents, rule_ids, pos_cnt, neg_cnt)
    zero_b1 = not np.any(np.asarray(b1))
    zero_b2 = not np.any(np.asarray(b2))
    b_eval_vals = [float(x) for x in np.asarray(b_eval)]
    nc = _build(prep, zero_b1, zero_b2, b_eval_vals)

    common = dict(
        thax0=np.ascontiguousarray(np.asarray(thax_table, np.float32)[0]),
        thax1=np.ascontiguousarray(np.asarray(thax_table, np.float32)[1]),
        W1=np.ascontiguousarray(np.asarray(W1, np.float32)),
        W2=np.ascontiguousarray(np.asarray(W2, np.float32)),
        b1=np.ascontiguousarray(np.asarray(b1, np.float32)),
        b2=np.ascontiguousarray(np.asarray(b2, np.float32)),
        sine_w=np.ascontiguousarray(np.asarray(sine_w, np.float32)),
        sine_b=np.ascontiguousarray(np.asarray(sine_b, np.float32)),
        w_eval=np.ascontiguousarray(np.asarray(w_eval, np.float32)),
    )
    in_maps = []
    for c in range(NC_):
        in_maps.append(dict(common,
                            gidx=prep["gidx"][c], tidx=prep["tidx"][c],
                            sval=prep["sval"][c], co=prep["co"][c]))
    trace = os.environ.get("KTRACE", "0") == "1"
    res = run_bass_kernel_spmd(nc, in_maps, core_ids=list(range(NC_)), trace=trace)
    if trace and res.exec_time_ns is not None:
        print(f"HW exec time: {res.exec_time_ns} ns")

    loss = np.zeros(M, np.float64)
    posOK = np.zeros(M, np.float64)
    negge = np.zeros(M, np.float64)
    for c in range(NC_):
        o = np.asarray(res.results[c]["out"], np.float64)
        loss += o[0:2]
        posOK += o[2:4]
        negge += o[4:6]
    negOK = prep["mneg_total"] - negge
    return np.stack([loss, posOK, negOK]).astype(np.float32)
